# revision 62
# baseline (speedup 1.0000x reference)
"""Trainium2 Bass kernel for nn_NUFFTLayerMultiChannelInitMixed.

Math: the reference's spread->FFT->filter->IFFT->energy pipeline is an exact
bilinear form in the (analytic) spectrum of the periodized Gaussians:

  ghat_n(k) = Cc * [A_n(k) cos(k x_n) - B_n(k) sin(k x_n)  ;  imag part]
  A = p_k + q_k cos(M x_n),  B = d_k sin(M x_n)

Energy per channel i (filter F_i(k) even in k, truncated at KT=32 modes --
tail is ~1e-4 relative):

  e_i[n] = T_i[n] - self_i[n]
  T_i[n] = sum_k wF_i (Re_n ReS + Im_n ImS)     (S = sum over the batch's points)
  self_i[n] = quadratic polynomial in cos/sin(M x_n)

Layout: quad-packed. The 128 partitions hold 4 groups x 32 k-modes; group
g = (batch b, half h) covers points n in [512h, 512h+512) of batch b (2
batches per core, data-parallel over 8 cores, no collectives). All trig
matrices are [128, 512] and are computed in BOTH [kappa, j] and [j, kappa]
layouts directly from two small phase matmuls (no PE transposes). bf16
trig/matmul inputs, fp32 PSUM accumulation. The per-group S-sums are halves;
a stream_shuffle partition swap + add merges partner halves.
"""

import numpy as np

try:
    import concourse.bass as bass
except ImportError:
    import sys
    sys.path.insert(0, "/opt/trn_rl_repo")
    import concourse.bass as bass

import concourse.bacc as bacc
import concourse.mybir as mybir
from concourse import tile
from concourse.bass_utils import run_bass_kernel_spmd
import ml_dtypes

F32 = mybir.dt.float32
BF16 = mybir.dt.bfloat16
AF = mybir.ActivationFunctionType
ALU = mybir.AluOpType

M = 2001
L = 2.0 * np.pi
TAU = 12.0 * (L / (2.0 * np.pi * M)) ** 2
KT = 32
B_FULL, N = 16, 1024
NCORES = 8
BPC = B_FULL // NCORES
MAGIC = 12582912.0          # 1.5 * 2^23: (u + MAGIC) - MAGIC = round(u)
PI = float(np.pi)

GIDX = np.arange(128) // KT
KIDX = np.arange(128) % KT


def _host_constants(shift0, shift1, amp0, amp1):
    """fp64 host-side tables -> tbl [128, 84] f32 (cstR|cstI|cstU|tpack-slot)
    + self-energy scalars."""
    k = np.arange(KT, dtype=np.float64)
    tau = float(TAU)
    p = np.exp(-tau * k * k)
    apl = np.exp(-tau * (k + M) ** 2)
    amn = np.exp(-tau * (k - M) ** 2)
    q = apl + amn
    d = apl - amn
    Cc = (M / L) * np.sqrt(4.0 * np.pi * tau)
    deconv2 = (np.pi / tau) * np.exp(2.0 * tau * k * k)
    mult1 = float(amp0) * (4.0 * np.pi) / (k * k + (1.0 * float(shift0)) ** 2)
    mult2 = float(amp1) * (4.0 * np.pi) / (k * k + (0.5 * float(shift1)) ** 2)
    w = np.full(KT, 2.0)
    w[0] = 1.0
    scale = 1.0 / ((2.0 * np.pi * M / L) * (2.0 * np.pi))
    pref = scale * Cc * Cc / M
    wF1 = w * deconv2 * mult1
    wF2 = w * deconv2 * mult2

    # ps24 cols: cos side: alpha-sums 0-3, beta-sums 4-7, C 8 (9-11 unused);
    # sin side: alpha 12-15, beta 16-19, S 20 (21-23 unused)
    r128 = np.arange(128)
    cstR = np.zeros((128, 24))
    cstI = np.zeros((128, 24))
    cstR[:, 8] = p[KIDX]
    cstR[r128, 0 + GIDX] = q[KIDX]
    cstR[r128, 16 + GIDX] = -d[KIDX]
    cstI[:, 20] = p[KIDX]
    cstI[r128, 12 + GIDX] = q[KIDX]
    cstI[r128, 4 + GIDX] = d[KIDX]

    cstU = np.zeros((128, 32))
    vals = [pref * p * wF1, pref * q * wF1, pref * p * wF2,
            pref * q * wF2, pref * d * wF1, pref * d * wF2]
    for c in range(6):
        cstU[r128, 6 * GIDX + c] = vals[c][KIDX]
    cstU[r128, 24 + 2 * GIDX + 0] = (-pref * d * wF1)[KIDX]
    cstU[r128, 24 + 2 * GIDX + 1] = (-pref * d * wF2)[KIDX]

    # tbl layout: 0:24 cstR | 24:48 cstI | 48:80 cstU | 80:96 tpack-slot |
    #             96:224 partner-perm P | 224:352 identity
    tbl = np.zeros((128, 352), dtype=np.float64)
    tbl[:, 0:24] = cstR
    tbl[:, 24:48] = cstI
    tbl[:, 48:80] = cstU
    # partner-sum permutation matrix: P[kappa, kappa'] = 1 iff same k-mode
    # and same batch (kappa' = kappa or its other-half partner kappa^32)
    r = np.arange(128)
    tbl[r, 96 + r] = 1.0
    tbl[r, 96 + (r ^ 32)] = 1.0
    tbl[r, 224 + r] = 1.0

    def selfsc(wF):
        return [float(pref * np.sum(wF * p * p)),
                float(pref * 2.0 * np.sum(wF * p * q)),
                float(pref * np.sum(wF * q * q)),
                float(pref * np.sum(wF * d * d))]

    return tbl.astype(np.float32), selfsc(wF1), selfsc(wF2)


def _v(tile_like, offset, dims):
    """Custom free-dim view of a tile AP: dims = [[step, count], ...]."""
    ap = tile_like[:]
    return bass.AP(ap.tensor, ap.offset + offset, [ap.ap[0]] + dims)


def _bc(col_ap, n):
    """Broadcast a [128, 1] AP along free dim to [128, n] (step 0)."""
    ap = col_ap[:] if not isinstance(col_ap, bass.AP) else col_ap
    return bass.AP(ap.tensor, ap.offset, [ap.ap[0], [0, n]])


def _build_program(sc1, sc2, debug=False):
    nc = bacc.Bacc(None, target_bir_lowering=False, debug=debug)
    xio_in = nc.declare_dram_parameter("xio", [8, 640], BF16, isOutput=False)
    tbl_in = nc.declare_dram_parameter("tbl", [128, 352], F32, isOutput=False)
    out_t = nc.declare_dram_parameter("out", [BPC, N, 2], F32, isOutput=True)

    with tile.TileContext(nc) as tc:
        import contextlib
        with contextlib.ExitStack() as ctx:
            pc = ctx.enter_context(tc.tile_pool(name="const", bufs=1))
            wp = ctx.enter_context(tc.tile_pool(name="work", bufs=1))
            ps_a = ctx.enter_context(tc.tile_pool(name="psa", bufs=1, space="PSUM"))
            ps_b = ctx.enter_context(tc.tile_pool(name="psb", bufs=1, space="PSUM"))
            ps_c = ctx.enter_context(tc.tile_pool(name="psc", bufs=1, space="PSUM"))
            ps_d = ctx.enter_context(tc.tile_pool(name="psd", bufs=1, space="PSUM"))
            ps_e = ctx.enter_context(tc.tile_pool(name="pse", bufs=1, space="PSUM"))

            # ---- inputs ----
            xio = pc.tile([8, 640], BF16, tag="xio")
            nc.sync.dma_start(xio[:], xio_in[:])
            tbl = pc.tile([128, 352], F32, tag="tbl")
            nc.sync.dma_start(tbl[:], tbl_in[:])
            X8 = lambda s: xio[:, s]          # [8, 0:512] t_hi|t_lo
            T8 = xio[:, 512:640]              # [8, 128]  group-masked kv

            # warm the Sin table during the DMA window
            scr1 = wp.tile([128, 1], F32, tag="scr1")
            nc.vector.memset(scr1[:], 0.0)
            nc.scalar.activation(scr1[:], scr1[:], AF.Sin, scale=1.0)

            # bf16 copy of the partner-perm matrix (exact: entries are 0/1)
            tblb = wp.tile([128, 128], BF16, tag="tblb")
            nc.vector.tensor_copy(tblb[:], tbl[:, 96:224])

            # ---- alpha/beta FIRST (gates the S-matmuls via W2; only needs tbl)
            # W2 [128, 48] = [alpha(16) | beta(16) | ones(16)]
            tpk = tbl[:, 80:96]
            u8 = wp.tile([128, 16], F32, tag="u8")
            nc.vector.tensor_scalar(u8[:], tpk, float(M), None, ALU.mult)
            rni8 = wp.tile([128, 16], F32, tag="rni8")
            nc.vector.tensor_scalar(rni8[:], u8[:], MAGIC, MAGIC, ALU.add, ALU.subtract)
            rneg8 = wp.tile([128, 16], F32, tag="rneg8")
            nc.vector.tensor_sub(rneg8[:], rni8[:], u8[:])
            W2 = wp.tile([128, 48], BF16, tag="W2")
            nc.gpsimd.memset(W2[:, 32:48], 1.0)
            nc.scalar.activation(W2[:, 16:32], rneg8[:], AF.Sin, scale=-2.0 * PI)
            h8 = wp.tile([128, 16], BF16, tag="h8")
            nc.scalar.activation(h8[:], rneg8[:], AF.Sin, scale=-PI)
            hh8 = wp.tile([128, 16], BF16, tag="hh8")
            nc.vector.tensor_mul(hh8[:], h8[:], h8[:])
            nc.vector.tensor_scalar(W2[:, 0:16], hh8[:], -2.0, 1.0,
                                    ALU.mult, ALU.add)
            alv = W2[:, 0:16]
            bev = W2[:, 16:32]

            # ---- phase matmuls ----
            pmn = ps_a.tile([128, 512], F32, tag="pmn")   # [j', 4 blocks x kappa]
            for jb in range(4):
                nc.tensor.matmul(pmn[:, 128 * jb:128 * jb + 128],
                                 X8(slice(128 * jb, 128 * jb + 128)), T8,
                                 start=True, stop=True)
            pmk = ps_b.tile([128, 512], F32, tag="pmk")   # [kappa, j]
            nc.tensor.matmul(pmk[:], T8, X8(slice(0, 512)), start=True, stop=True)

            # ---- range reduction + trig, n-layout ----
            # rneg = round(u) - u in [-.5, .5]; sin(2pi u) = Sin(-2pi*rneg)
            # cos(2pi u) = 1 - 2*Sin(-pi*rneg)^2
            # stage PSUM->SBUF on the idle scalar engine: vector's tensor_scalar
            # runs 2x from SBUF vs 1x from PSUM
            pmnS = wp.tile([128, 512], F32, tag="pmnS")
            nc.scalar.copy(pmnS[:], pmn[:])
            rniN = wp.tile([128, 512], F32, tag="rniN")
            nc.vector.tensor_scalar(rniN[:], pmnS[:], MAGIC, MAGIC, ALU.add, ALU.subtract)
            rnegN = wp.tile([128, 512], F32, tag="rnegN")
            nc.vector.tensor_sub(rnegN[:], rniN[:], pmnS[:])
            snctN = wp.tile([128, 1024], BF16, tag="snctN")  # [sin | cos]
            nc.scalar.activation(snctN[:, 0:512], rnegN[:], AF.Sin, scale=-2.0 * PI)
            hN = wp.tile([128, 512], BF16, tag="hN")
            nc.scalar.activation(hN[:], rnegN[:], AF.Sin, scale=-PI)
            hhN = wp.tile([128, 512], BF16, tag="hhN")
            nc.vector.tensor_mul(hhN[:], hN[:], hN[:])
            nc.vector.tensor_scalar(snctN[:, 512:1024], hhN[:], -2.0, 1.0,
                                    ALU.mult, ALU.add)

            # ---- k-layout ----
            pmkS = wp.tile([128, 512], F32, tag="pmkS")
            nc.scalar.copy(pmkS[:], pmk[:])
            rniK = wp.tile([128, 512], F32, tag="rniK")
            nc.vector.tensor_scalar(rniK[:], pmkS[:], MAGIC, MAGIC, ALU.add, ALU.subtract)
            rnegK = wp.tile([128, 512], F32, tag="rnegK")
            nc.vector.tensor_sub(rnegK[:], rniK[:], pmkS[:])
            snctK = wp.tile([128, 1024], BF16, tag="snctK")
            nc.scalar.activation(snctK[:, 0:512], rnegK[:], AF.Sin, scale=-2.0 * PI)
            hK = wp.tile([128, 512], BF16, tag="hK")
            nc.scalar.activation(hK[:], rnegK[:], AF.Sin, scale=-PI)
            hhK = wp.tile([128, 512], BF16, tag="hhK")
            nc.vector.tensor_mul(hhK[:], hK[:], hK[:])
            nc.vector.tensor_scalar(snctK[:, 512:1024], hhK[:], -2.0, 1.0,
                                    ALU.mult, ALU.add)

            # ---- S-side matmuls: ps24 [kappa, 12+12]; moving = W2 cols
            # {4jb..+3 (alpha), 16+4jb..+3 (beta), 32+4jb..+3 (ones)} ----
            ps24 = ps_c.tile([128, 24], F32, tag="ps24")
            wmov = lambda jb: _v(W2, 4 * jb, [[16, 3], [1, 4]])
            for jb in range(4):
                nc.tensor.matmul(ps24[:, 12:24], snctN[:, 128 * jb:128 * jb + 128],
                                 wmov(jb), start=(jb == 0), stop=(jb == 3))
            for jb in range(4):
                nc.tensor.matmul(ps24[:, 0:12], snctN[:, 512 + 128 * jb:512 + 128 * jb + 128],
                                 wmov(jb), start=(jb == 0), stop=(jb == 3))

            # ---- S combine on gpsimd (vector is busy with trig folds); the
            # cst muls read ps24 -> must stage PSUM->SBUF first (Pool can't
            # read PSUM)
            ps24s = wp.tile([128, 24], F32, tag="ps24s")
            nc.vector.tensor_copy(ps24s[:], ps24[:])
            scrAB = wp.tile([128, 48], F32, tag="scrAB")
            srsi = wp.tile([128, 2], F32, tag="srsi")
            nc.gpsimd.tensor_mul(scrAB[:, 0:24], tbl[:, 0:24], ps24s[:])
            nc.gpsimd.tensor_mul(scrAB[:, 24:48], tbl[:, 24:48], ps24s[:])
            nc.vector.tensor_reduce(srsi[:, 0:1], scrAB[:, 0:24],
                                    mybir.AxisListType.X, ALU.add)
            nc.vector.tensor_reduce(srsi[:, 1:2], scrAB[:, 24:48],
                                    mybir.AxisListType.X, ALU.add)
            # partner-half merge via permutation matmul (stream_shuffle can't
            # cross 32-partition blocks). bf16 hi/lo split keeps the matmul
            # single-pass (fp32 stationary loads twice as LOW|HIGH) while
            # staying fp32-exact to ~1e-5: srsi = hi + lo, P is 0/1-exact.
            srhl = wp.tile([128, 4], BF16, tag="srhl")
            nc.vector.tensor_copy(srhl[:, 0:2], srsi[:])
            nc.vector.tensor_sub(srhl[:, 2:4], srsi[:], srhl[:, 0:2])
            psr = ps_e.tile([128, 4], F32, tag="psr")
            nc.tensor.matmul(psr[:], tblb[:], srhl[:], start=True, stop=True)
            psrs = wp.tile([128, 4], F32, tag="psrs")
            nc.vector.tensor_copy(psrs[:], psr[:])
            srsiF = wp.tile([128, 2], F32, tag="srsiF")
            nc.vector.tensor_add(srsiF[:], psrs[:, 0:2], psrs[:, 2:4])

            # ---- U vectors [128, 24] bf16 (g-major, 6 cols per group) ----
            cstu = lambda off, dims: _v(tbl, 48 + off, dims)
            UC = wp.tile([128, 24], BF16, tag="UC")
            US = wp.tile([128, 24], BF16, tag="US")
            pq = [[6, 4], [1, 4]]   # cols 6g+0..3
            dw = [[6, 4], [1, 2]]   # cols 6g+4..5
            nc.vector.tensor_tensor(_v(UC, 0, pq), cstu(0, pq),
                                    _bc(srsiF[:, 0:1], 16), ALU.mult)
            nc.vector.tensor_tensor(_v(UC, 4, dw), cstu(4, dw),
                                    _bc(srsiF[:, 1:2], 8), ALU.mult)
            nc.gpsimd.tensor_tensor(_v(US, 0, pq), cstu(0, pq),
                                    _bc(srsiF[:, 1:2], 16), ALU.mult)
            nc.gpsimd.tensor_tensor(_v(US, 4, dw), cstu(24, [[2, 4], [1, 2]]),
                                    _bc(srsiF[:, 0:1], 8), ALU.mult)

            # ---- T-side matmuls: pt [j, 4 blocks x 24] ----
            pt = ps_d.tile([128, 96], F32, tag="pt")
            for jb in range(4):
                nc.tensor.matmul(pt[:, 24 * jb:24 * jb + 24],
                                 snctK[:, 512 + 128 * jb:512 + 128 * jb + 128],
                                 UC[:], start=True, stop=False)
                nc.tensor.matmul(pt[:, 24 * jb:24 * jb + 24],
                                 snctK[:, 128 * jb:128 * jb + 128],
                                 US[:], start=False, stop=True)

            # ---- combine + self energy; ch1 on vector, ch2 on gpsimd ----
            sT = wp.tile([128, 96], F32, tag="sT")
            nc.vector.tensor_copy(sT[:], pt[:])
            aa = wp.tile([128, 16], F32, tag="aa")
            nc.vector.tensor_mul(aa[:], alv, alv)
            bb = wp.tile([128, 16], F32, tag="bb")
            nc.gpsimd.tensor_mul(bb[:], bev, bev)
            # ev col = 8g + 2jb + c  (g = 2b + h) so the out DMA gets
            # contiguous c-pairs innermost
            ev = wp.tile([128, 32], F32, tag="ev")

            # self-energy polynomials on vector (STT illegal on Pool)
            jg = lambda c: _v(sT, c, [[24, 4], [6, 4]])   # (jb, g) view of col c
            saccs = []
            for i, sc in enumerate((sc1, sc2)):
                sacc = wp.tile([128, 16], F32, tag=f"sacc_{i}")
                nc.vector.tensor_scalar(sacc[:], alv, sc[1], sc[0], ALU.mult, ALU.add)
                nc.vector.scalar_tensor_tensor(sacc[:], aa[:], sc[2], sacc[:],
                                               ALU.mult, ALU.add)
                nc.vector.scalar_tensor_tensor(sacc[:], bb[:], sc[3], sacc[:],
                                               ALU.mult, ALU.add)
                saccs.append(sacc)
            for i, (c0, c1, c4, eng) in enumerate(
                    ((0, 1, 4, nc.vector), (2, 3, 5, nc.gpsimd))):
                m1 = wp.tile([128, 16], F32, tag=f"m1_{i}")
                eng.tensor_mul(m1[:], alv, jg(c1))
                s1 = wp.tile([128, 16], F32, tag=f"s1_{i}")
                eng.tensor_add(s1[:], jg(c0), m1[:])
                m2 = wp.tile([128, 16], F32, tag=f"m2_{i}")
                eng.tensor_mul(m2[:], bev, jg(c4))
                tt = wp.tile([128, 16], F32, tag=f"tt_{i}")
                eng.tensor_add(tt[:], s1[:], m2[:])
                eng.tensor_sub(_v(ev, i, [[2, 4], [8, 4]]), tt[:], saccs[i][:])

            # ---- output: PE-transpose ev -> evT[k, 2j+c] so the DMA writes
            # 16 contiguous 1KB runs instead of 2048 8-byte scatters
            evT = ps_e.tile([16, 256], F32, tag="evT")
            for c in range(2):
                nc.tensor.transpose(_v(evT, c, [[2, 128]]),
                                    _v(ev, c, [[2, 16]]), tbl[:, 224:352])
            evTs = wp.tile([16, 256], F32, tag="evTs")
            nc.vector.tensor_copy(evTs[:], evT[:])
            # dram elem addr = 2048b+2n+c = 256*(8b+4h+jb) + (2j+c) = 256k + f
            out_ap = bass.AP(out_t[:].tensor, 0, [[256, 16], [1, 256]])
            nc.sync.dma_start(out_ap, evTs[:])
    return nc


def _prepare(x, shift0, shift1, amp0, amp1):
    x = np.asarray(x, dtype=np.float32)
    tbl, sc1, sc2 = _host_constants(shift0.reshape(-1)[0], shift1.reshape(-1)[0],
                                    amp0.reshape(-1)[0], amp1.reshape(-1)[0])
    nc = _build_program(sc1, sc2)

    t = (x.astype(np.float64) / (2.0 * np.pi)).astype(np.float32)   # [16, 1024]
    t_hi = t.astype(ml_dtypes.bfloat16)
    t_lo = (t.astype(np.float64) - t_hi.astype(np.float64)).astype(np.float32)

    kv = np.arange(KT, dtype=np.float64)
    T8 = np.zeros((8, 128), dtype=np.float64)
    for pp in range(8):
        sel = GIDX == (pp % 4)
        T8[pp, sel] = kv[KIDX[sel]]

    in_maps = []
    for c in range(NCORES):
        tb = t[2 * c:2 * c + 2]
        xio = np.zeros((8, 640), dtype=ml_dtypes.bfloat16)
        for g in range(4):
            b, h = g >> 1, g & 1
            xio[g, 0:512] = t_hi[2 * c + b, 512 * h:512 * h + 512]
            xio[4 + g, 0:512] = t_lo[2 * c + b, 512 * h:512 * h + 512].astype(ml_dtypes.bfloat16)
        xio[:, 512:640] = T8.astype(ml_dtypes.bfloat16)
        tblc = tbl.copy()
        for g in range(4):
            b, h = g >> 1, g & 1
            for jb in range(4):
                tblc[:, 80 + 4 * jb + g] = tb[b, 512 * h + 128 * jb:512 * h + 128 * jb + 128]
        in_maps.append({"xio": xio, "tbl": tblc})
    return nc, in_maps


def kernel(x, shift0, shift1, amp0, amp1):
    nc, in_maps = _prepare(x, shift0, shift1, amp0, amp1)
    nc.finalize()
    res = run_bass_kernel_spmd(nc, in_maps, list(range(NCORES)))
    out = np.concatenate([res.results[c]["out"] for c in range(NCORES)], axis=0)
    return out.astype(np.float32)
